# revision 2
# baseline (speedup 1.0000x reference)
"""Dot-product stereo cost volume on 8 Trainium2 NeuronCores.

cost[b, d, y, x] = sum_c left[b,c,y,x] * right[b,c,y,x-d], zeros where x-d < 0.
Shapes: left/right [4, 128, 192, 640] fp32, D = 96 -> out [4, 96, 192, 640] fp32.

Strategy (v3, HW-ablation-driven)
---------------------------------
Sharding: 8 cores <- (b, y-half): core k handles batch k//2, rows 96*(k%2)..+96.

Per row y the math is a banded Gram matrix G_y[x', x] = sum_c R[c,x'] L[c,x]
with cost[d, y, x] = G_y[x-d, x].  Five M=128 x'-tiles per row, one
full-array fp32 matmul each (measured ~130 ns/matmul warm — fp32 streams far
faster than the 4 cyc/col book number):
  ps[p, f] = G[128t + p, 128t + f],  f in [0, 223)
(L zero-padded to 736 cols so the t=4 window is in range).

The diagonal band (d = x - x' in [0,96)) is compacted at 64-partition
granularity and cast fp32->bf16 on the way out of PSUM (only DVE and ACT can
read PSUM):
  DVE: st[0:64,  y, t, :] = ps[0:64,   0:159]
  ACT: st[64:128, y, t, :] = ps[64:128, 64:223]
Host gather: x' = x-d, t = x'//128, p = x'%128, col = 159 t + (p%64) + d;
upcast bf16->fp32.  bf16 stores are a pure 2^-9 relative error (gate is
2e-2); inputs must stay fp32 because input rounding creates absolute error
that the rel-err gate punishes at near-zero outputs.

DMA routing (the part that actually mattered on HW):
- left+right are host-stacked into one tensor and loaded with ONE DMA per
  8-row group on the SP HWDGE ring.  Loads must not share an engine queue
  with the ACT copies (strict FIFO: copy instructions ahead of a dma_start
  trigger stall the load) and merging halves the ring transactions.
- stores (bf16 stage, one 3.2 MB DMA per group) issue from the ACT ring,
  naturally paced behind the copies.  Loads+stores on one ring, or
  fine-grained stores, collapse throughput (read/write mixing).
Measured ~127-230 us/core depending on machine load (baseline 229-276 us
under the same conditions; ablations: PE 63 us, loads 90 us at ~700 GB/s,
split copies ~139 us, stores overlap-free).
"""

import sys

if "/opt/trn_rl_repo" not in sys.path:
    sys.path.insert(0, "/opt/trn_rl_repo")

import numpy as np

B, C, H, W = 4, 128, 192, 640
D = 96
HSH = H // 2          # rows per core
MT = 128              # x'-tile height
NT = W // MT          # x'-tiles per row = 5
FW = MT + D - 1       # psum cols per tile = 223
SW = 64 + D - 1       # stage cols per tile = 159
RW = NT * SW          # per-row stage width = 795
WPAD = 736            # L padded so the t=4 window [512, 735) is in range
YB = 8                # rows per load/store group

_compiled = None


def _build(repeat=1, yb=YB, lbufs=2, sbufs=3, pbufs=6,
           load_eng="sync", store_eng="scalar"):
    import contextlib
    import concourse.bacc as bacc
    import concourse.tile as tile
    import concourse.mybir as mybir

    nc = bacc.Bacc("TRN2", target_bir_lowering=False, debug=False, num_devices=8)
    f32 = mybir.dt.float32
    bf16 = mybir.dt.bfloat16

    # left and right stacked on the host: lr[c, 0] = left, lr[c, 1] = right
    lr_ap = nc.dram_tensor("lr", [C, 2, HSH, W], f32, kind="ExternalInput").ap()
    scr_ap = nc.dram_tensor(
        "scr", [HSH // yb, 128, yb * RW], bf16, kind="ExternalOutput"
    ).ap()

    with tile.TileContext(nc) as tc:
        with (
            tc.tile_pool(name="lrpool", bufs=lbufs) as lrpool,
            tc.tile_pool(name="stage", bufs=sbufs) as stage_pool,
            tc.tile_pool(name="psum", bufs=pbufs, space="PSUM") as psum_pool,
        ):
            rep_ctx = (
                tc.For_i(0, repeat, 1) if repeat > 1 else contextlib.nullcontext()
            )
            with rep_ctx:
                for y0 in range(0, HSH, yb):
                    lrt = lrpool.tile(
                        [128, 2 * yb * WPAD], f32, name=f"lrt_{y0}", tag="lrt"
                    )
                    lrt4 = lrt.rearrange("c (s y w) -> c s y w", s=2, y=yb)
                    # one DMA per y-group for both tensors, on the SP ring
                    getattr(nc, load_eng).dma_start(
                        lrt4[:, :, :, 0:W], lr_ap[:, :, y0 : y0 + yb, :]
                    )
                    # zero-pad tail of the left rows (matmul windows read it)
                    nc.vector.memset(lrt4[:, 0, :, W:WPAD], 0.0)

                    st = stage_pool.tile(
                        [128, yb * RW], bf16, name=f"st_{y0}", tag="st"
                    )
                    stv = st.rearrange("p (y t f) -> p y t f", y=yb, t=NT)
                    for yi in range(yb):
                        lof = yi * WPAD               # left row base
                        rof = yb * WPAD + yi * WPAD   # right row base
                        for t in range(NT):
                            ps = psum_pool.tile(
                                [128, FW], f32, name=f"ps_{y0}_{yi}_{t}", tag="ps"
                            )
                            nc.tensor.matmul(
                                ps[:, :],
                                lhsT=lrt[:, rof + MT * t : rof + MT * t + MT],
                                rhs=lrt[:, lof + MT * t : lof + MT * t + FW],
                                start=True,
                                stop=True,
                            )
                            nc.vector.tensor_copy(
                                stv[0:64, yi, t, :], ps[0:64, 0:SW]
                            )
                            nc.scalar.copy(
                                stv[64:128, yi, t, :], ps[64:128, 64 : 64 + SW]
                            )
                    getattr(nc, store_eng).dma_start(scr_ap[y0 // yb], st[:])

    nc.compile()
    return nc


def _host_index():
    """idx[d, x] -> flat offset into a scr row (= [128 p, RW]) holding G[x-d, x].

    st[p, 159 t + f] = G[128 t + p, 128 t + 64 (p//64) + f]; for (d, x):
    x' = x - d, t = x'//128, p = x'%128, col = 159 t + (p%64) + d (< 159
    always).  Valid only where x >= d; mask handles the rest.
    """
    d = np.arange(D)[:, None]
    x = np.arange(W)[None, :]
    xp = np.maximum(x - d, 0)
    t = xp // MT
    p = xp - MT * t
    col = SW * t + (p % 64) + d
    idx = p * RW + col
    mask = x >= d
    return idx.astype(np.int64), mask


def kernel(left, right, num_disparities):
    global _compiled
    left = np.asarray(left)
    right = np.asarray(right)
    assert int(num_disparities) == D
    assert left.shape == (B, C, H, W) and right.shape == (B, C, H, W)

    if _compiled is None:
        _compiled = _build()
    nc = _compiled

    from concourse.bass_utils import run_bass_kernel_spmd

    in_maps = []
    for k in range(8):
        b, hh = k // 2, k % 2
        sl = slice(HSH * hh, HSH * hh + HSH)
        in_maps.append(
            {"lr": np.ascontiguousarray(
                np.stack([left[b, :, sl, :], right[b, :, sl, :]], axis=1))}
        )

    res = run_bass_kernel_spmd(nc, in_maps, list(range(8)))

    idx, mask = _host_index()
    out = np.zeros((B, D, H, W), dtype=np.float32)
    for k in range(8):
        b, hh = k // 2, k % 2
        raw = np.asarray(res.results[k]["scr"])
        scr = (
            raw.reshape(-1, 128, raw.shape[2] // RW, RW)
            .swapaxes(1, 2)
            .reshape(HSH, 128 * RW)
        )
        gathered = scr[:, idx.ravel()].astype(np.float32).reshape(HSH, D, W)
        gathered *= mask[None, :, :]
        out[b, :, HSH * hh : HSH * hh + HSH, :] = gathered.transpose(1, 0, 2)
    return out
